# revision 50
# baseline (speedup 1.0000x reference)
"""Causal self-attention (B=2, T=2048, D=1024, H=16) on 8 trn2 NeuronCores.

Sharding: core = b*4 + g  (b = batch 0/1, g = head-group of 4 heads).
Each core computes its 4 heads' attention for its batch plus the partial
output projection (Wproj rows for those heads); host sums the 4 bf16
partials per batch (the tensor-parallel all-reduce).

Design (289us v1 baseline -> 175us):
  - all matmul operands bf16 (host ships bf16): halves input DMA and
    avoids the f32r 4-cyc/row penalty on small-free-dim diagonal blocks.
  - stage-1 x DMA is (chunk, kt)-sliced so QKV matmuls start after
    ~2.5MB instead of ~11MB.
  - softmax exp is the scalar(ACT)-engine drumbeat ((N+352)/1.2 ns per
    call, ~75us total): exp for the first NPRE blocks is pre-computed
    during stage 1 (the scalar engine idles there otherwise), and the
    remaining live blocks' scores+exp are hoisted ahead of their AV
    consumption so the scalar engine never idles and the in-order PE
    queue never waits on a just-issued exp. ex tiles live in an
    EXB-deep bf16 ring.
  - softmax normalize: sumexp rows are staged to SBUF, stream-transposed
    (32x32 blocks) so tokens land on partitions, builtin reciprocal runs
    on a [64,16] strided view (16 elems/partition: 0.1us vs 3.3us for
    the [1,512] row shape - DVE reciprocal cost is per-partition-elems),
    transposed back, cast to bf16, then broadcast to 128 partitions via
    a K=1 ones-matmul (PE) and applied by DVE muls. The PE-side
    broadcast+muls are DEFERRED into the next (c,p) unit's block stream
    so the PE never stalls on the DVE chain. (gpsimd partition_broadcast
    and the custom-DVE reciprocal_approx_fast both corrupt on HW when
    mixed with builtin ops - verified; both avoided.)
  - proj PSUM tiles share the scores tag so everything fits in 8 banks:
    stage1 = pq(2) + pv(2) + sc(4); attention = sc(4) + av(4).
  - proj output tiles are cast to bf16 on the scalar engine (ACT Copy
    needs no table) and DMA'd as bf16 partials.

Per-core layouts:
  xT      [1024, 2048] bf16  x[b] transposed (host)    -> sbuf xt  [128, 8*2048]
  wqkv    [1024, 768]  bf16  [q 4h | k 4h | v 4h] cols -> sbuf w   [128, 8*768]
  wproj   [128, 2048]  bf16  pair-major Wproj rows
  mask    [128, 256]   bf16  causal tri x2 copies
  out     [2048, 1024] bf16  partial projection output

qT/kT pair tiles [128, 512] per (qk,p,c): head-even rows 0:64, head-odd
rows 64:128 (K=64 matmuls at base partition 0/64).

v tile per key-block jb is [128, 386]:
  even pair half (65 cols):  [v_h (64) | ones (1)]        -> AV rows 0:64, sumexp row 64
  odd  pair half (128 cols): [ones | zeros*63 | v_h (64)] -> sumexp row 0, AV rows 64:128
"""

import numpy as np

B, T, D, H, DH = 2, 2048, 1024, 16, 64
HPG = 4          # heads per group (per core)
NKT = D // 128   # 8 contraction tiles over D
NTT = T // 128   # 16 tiles over T (also key blocks)
NC_ = 4          # 4 i-chunks of 512 queries
VS = 386         # per-jb v-tile stride: 65 + 128 + 65 + 128
SCALE = 1.0 / np.sqrt(DH)

EXB = 43         # ex-tile ring depth (bf16 [128,1024] tiles, 2KB/partition)
NPRE = 37        # blocks pre-scored+exp'd during stage 1 (must be < EXB)
HOIST_GAP = EXB - 6

_PROG = None


def _build_program():
    from contextlib import ExitStack
    from concourse import bacc, mybir, tile

    f32 = mybir.dt.float32
    bf16 = mybir.dt.bfloat16
    Exp = mybir.ActivationFunctionType.Exp

    nc = bacc.Bacc(
        "TRN2", target_bir_lowering=False, debug=False, enable_asserts=False,
        num_devices=8,
    )
    xT_d = nc.dram_tensor("xT", [D, T], bf16, kind="ExternalInput").ap()
    wqkv_d = nc.dram_tensor("wqkv", [D, 3 * HPG * DH], bf16, kind="ExternalInput").ap()
    wproj_d = nc.dram_tensor("wproj", [128, 2 * D], bf16, kind="ExternalInput").ap()
    mask_d = nc.dram_tensor("mask", [128, 256], bf16, kind="ExternalInput").ap()
    vinit_d = nc.dram_tensor("vinit", [128, NTT * 130], bf16, kind="ExternalInput").ap()
    ones_d = nc.dram_tensor("ones", [128, 128], bf16, kind="ExternalInput").ap()
    out_d = nc.dram_tensor("out", [T, D], bf16, kind="ExternalOutput").ap()

    # global block order: (c, p, jb) — ex-ring consumption order
    blocks = [(c, p, jb)
              for c in range(NC_) for p in range(2) for jb in range(4 * c + 4)]

    with tile.TileContext(nc) as tc, ExitStack() as ctx:
        # ---- persistent pools -------------------------------------------
        const_pool = ctx.enter_context(tc.tile_pool(name="const", bufs=1))
        qk_pool = ctx.enter_context(tc.tile_pool(name="qk", bufs=1))
        v_pool = ctx.enter_context(tc.tile_pool(name="v", bufs=1))
        att_pool = ctx.enter_context(tc.tile_pool(name="att", bufs=1))
        exp_pool = ctx.enter_context(tc.tile_pool(name="exp", bufs=EXB))
        norm_pool = ctx.enter_context(tc.tile_pool(name="norm", bufs=2))
        ot_pool = ctx.enter_context(tc.tile_pool(name="ot", bufs=2))
        # scores psum: [128,1024] f32 = 2 banks, bufs=2 -> 4 banks.
        # proj pp tiles borrow this tag.
        psc_pool = ctx.enter_context(tc.tile_pool(name="psc", bufs=2, space="PSUM"))

        mask_sb = const_pool.tile([128, 256], bf16, tag="mask")
        wproj_sb = const_pool.tile([128, 2 * D], bf16, tag="wproj")
        ones_sb = const_pool.tile([128, 128], bf16, tag="ones")
        # persistent staging for the transpose-recip (filler rows must be
        # initialized once: the stream transposes read the full tile)
        st_p = const_pool.tile([64, 512], f32, tag="stp")
        rc_p = const_pool.tile([64, 512], f32, tag="rcp")
        nc.vector.memset(st_p[:], 1.0)
        nc.vector.memset(rc_p[:], 1.0)

        qk_t = {}
        for qk in range(2):
            for p in range(2):
                for c in range(NC_):
                    qk_t[qk, p, c] = qk_pool.tile(
                        [128, 512], bf16, tag=f"qk{qk}{p}{c}",
                        name=f"qkt{qk}{p}{c}")
        v_t = [v_pool.tile([128, VS], bf16, tag=f"v{jb}", name=f"vt{jb}")
               for jb in range(NTT)]
        att_t = {}
        for p in range(2):
            for c in range(NC_):
                att_t[p, c] = att_pool.tile([128, 512], bf16, tag=f"att{p}{c}",
                                            name=f"attt{p}{c}")

        # ---- helpers -----------------------------------------------------
        ex_tiles = {}     # block index -> ex tile
        sc_emitted = 0    # blocks whose scores+exp are emitted

        mk2 = mask_sb.rearrange("p (h i) -> p h i", h=2)

        def emit_sc_exp(bi):
            """scores matmul pair + exp (+ causal mask) for block bi."""
            c, p, jb = blocks[bi]
            r = jb - 4 * c
            off = 128 * r if r > 0 else 0
            sc = psc_pool.tile([128, 1024], f32, tag="sc", name="sc")
            kt_tile = qk_t[1, p, jb // 4]
            q_tile = qk_t[0, p, c]
            for par in range(2):
                rows = slice(par * 64, par * 64 + 64)
                nc.tensor.matmul(
                    sc[:, par * 512 + off:par * 512 + 512],
                    lhsT=kt_tile[rows, (jb % 4) * 128:(jb % 4) * 128 + 128],
                    rhs=q_tile[rows, off:512],
                    start=True, stop=True,
                )
            ex = exp_pool.tile([128, 1024], bf16, tag="ex", name="ex")
            sc2 = sc.rearrange("p (h i) -> p h i", h=2)
            ex2 = ex.rearrange("p (h i) -> p h i", h=2)
            nc.scalar.activation(ex2[:, :, off:512], sc2[:, :, off:512],
                                 Exp, scale=float(SCALE))
            if r >= 0:
                nc.gpsimd.tensor_mul(
                    ex2[:, :, off:off + 128],
                    ex2[:, :, off:off + 128], mk2[:],
                )
            ex_tiles[bi] = ex

        # ---- stage 1: QKV projection with interleaved pre-scores ---------
        with (
            tc.tile_pool(name="xt", bufs=1) as xt_pool,
            tc.tile_pool(name="wq", bufs=1) as wq_pool,
            tc.tile_pool(name="pq", bufs=2, space="PSUM") as pq_pool,
            tc.tile_pool(name="pv", bufs=2, space="PSUM") as pv_pool,
        ):
            xt_sb = xt_pool.tile([128, NKT * T], bf16, tag="xt")
            w_sb = wq_pool.tile([128, NKT * 768], bf16, tag="w")
            vst = xt_pool.tile([128, NTT * 130], bf16, tag="vst")

            # DMA order: mask first (pre-block masks), w + x(c0) slices
            # interleaved, then vinit/wproj, then x(c1..c3).
            nc.sync.dma_start(mask_sb[:], mask_d[:])
            # weights ride the Activation HWDGE queue family, x rides SP:
            # the critical first-chunk set transfers on both in parallel
            for kt in range(NKT):
                nc.scalar.dma_start(
                    w_sb[:, kt * 768:(kt + 1) * 768],
                    wqkv_d[kt * 128:(kt + 1) * 128, :],
                )
                nc.sync.dma_start(
                    xt_sb[:, kt * T:kt * T + 512],
                    xT_d[kt * 128:(kt + 1) * 128, 0:512],
                )
            nc.scalar.dma_start(vst[:], vinit_d[:])
            nc.scalar.dma_start(wproj_sb[:], wproj_d[:])
            nc.scalar.dma_start(ones_sb[:], ones_d[:])
            for c in range(1, NC_):
                for kt in range(NKT):
                    nc.sync.dma_start(
                        xt_sb[:, kt * T + c * 512:kt * T + (c + 1) * 512],
                        xT_d[kt * 128:(kt + 1) * 128, c * 512:(c + 1) * 512],
                    )

            # v-tile static cols (ones/zeros) from vinit
            vst3 = vst.rearrange("p (j q y) -> p j q y", j=NTT, q=2)
            for jb in range(NTT):
                vt2 = v_t[jb].rearrange("p (q y) -> p q y", q=2)
                nc.gpsimd.tensor_copy(vt2[:, :, 64:129], vst3[:, jb, :, :])

            # q/k projection, c-major; 1 pre-block per psum tile
            for c in range(NC_):
                for p in range(2):
                    for qk in range(2):
                        ps = pq_pool.tile([128, 512], f32, tag="pq")
                        for kt in range(NKT):
                            nc.tensor.matmul(
                                ps[:],
                                lhsT=w_sb[:, kt * 768 + qk * 256 + p * 128:
                                          kt * 768 + qk * 256 + p * 128 + 128],
                                rhs=xt_sb[:, kt * T + c * 512:
                                          kt * T + c * 512 + 512],
                                start=(kt == 0), stop=(kt == NKT - 1),
                            )
                        nc.vector.tensor_copy(qk_t[qk, p, c][:], ps[:])
                        # pre-blocks of chunk <= c-1 are safe (their q/k
                        # copies are emitted); up to 2 per pq tile keeps
                        # the scalar engine fed without outpacing it
                        n = 0
                        while (sc_emitted < NPRE and n < 2
                               and blocks[sc_emitted][0] < c):
                            emit_sc_exp(sc_emitted)
                            sc_emitted += 1
                            n += 1

            # v projection: 2 token-tiles per [128,512] psum tile;
            # 2 pre-blocks per pair
            for tp in range(NTT // 2):
                ps = pv_pool.tile([128, 512], f32, tag="pv")
                for half in range(2):
                    tt = 2 * tp + half
                    for kt in range(NKT):
                        nc.tensor.matmul(
                            ps[:, half * 256:half * 256 + 256],
                            lhsT=xt_sb[:, kt * T + tt * 128:kt * T + tt * 128 + 128],
                            rhs=w_sb[:, kt * 768 + 512:kt * 768 + 768],
                            start=(kt == 0), stop=(kt == NKT - 1),
                        )
                for half in range(2):
                    tt = 2 * tp + half
                    for p in range(2):
                        base = p * 193
                        nc.vector.tensor_copy(
                            v_t[tt][:, base:base + 64],
                            ps[:, half * 256 + (2 * p) * 64:
                               half * 256 + (2 * p) * 64 + 64],
                        )
                        nc.vector.tensor_copy(
                            v_t[tt][:, base + 129:base + 193],
                            ps[:, half * 256 + (2 * p + 1) * 64:
                               half * 256 + (2 * p + 1) * 64 + 64],
                        )
                n = 0
                while sc_emitted < NPRE and n < 2:
                    emit_sc_exp(sc_emitted)
                    sc_emitted += 1
                    n += 1

        # ---- stage 2+3: attention with hoisted scores + interleaved proj -
        def emit_proj(c):
            for tt in range(4 * c, 4 * c + 4):
                pp = psc_pool.tile([128, 1024], f32, tag="sc", name="pp")
                for ch in range(2):
                    for p in range(2):
                        nc.tensor.matmul(
                            pp[:, ch * 512:ch * 512 + 512],
                            lhsT=att_t[p, tt // 4][:, (tt % 4) * 128:
                                                   (tt % 4) * 128 + 128],
                            rhs=wproj_sb[:, p * D + ch * 512:p * D + ch * 512 + 512],
                            start=(p == 0), stop=(p == 1),
                        )
                ot = ot_pool.tile([128, D], bf16, tag="ot", bufs=3)
                nc.scalar.copy(ot[:], pp[:])
                # alternate the two HWDGE queue families so output stores
                # don't serialize behind one queue's backlog
                eng = nc.sync if tt % 2 == 0 else nc.scalar
                eng.dma_start(out_d[tt * 128:tt * 128 + 128, :], ot[:])

        with tc.tile_pool(name="pav", bufs=1, space="PSUM") as pav_pool:
            av_i = 0
            pending_norm = []  # deferred broadcast+mul: (c, p, av_e, av_o, rec)

            def flush_norm():
                c0, p0, av_e0, av_o0, rec0 = pending_norm.pop(0)
                # broadcast 1/sumexp rows to all 128 partitions via a K=1
                # ones-matmul (PE; recip long done by the time PE gets here)
                rb = psc_pool.tile([128, 1024], f32, tag="sc", name="rb")
                nc.tensor.matmul(
                    rb[:, 0:512], lhsT=ones_sb[0:1, :],
                    rhs=rec0[0:1, 0:512], start=True, stop=True,
                )
                nc.tensor.matmul(
                    rb[:, 512:1024], lhsT=ones_sb[32:33, :],
                    rhs=rec0[32:33, 512:1024], start=True, stop=True,
                )
                # DVE reads only one PSUM operand per op: stage rb in SBUF
                rbs = norm_pool.tile([128, 1024], f32, tag="rbs", name="rbs")
                nc.vector.tensor_copy(rbs[:], rb[:])
                nc.vector.tensor_mul(
                    att_t[p0, c0][0:64, :], av_e0[0:64, :], rbs[0:64, 0:512])
                nc.vector.tensor_mul(
                    att_t[p0, c0][64:128, :], av_o0[64:128, :],
                    rbs[64:128, 512:1024])

            for c in range(NC_):
                njb = 4 * c + 4
                for p in range(2):
                    av_e = pav_pool.tile([128, 512], f32, tag=f"avE{p}",
                                         name="av_e")
                    av_o = pav_pool.tile([128, 512], f32, tag=f"avO{p}",
                                         name="av_o")
                    for jb in range(njb):
                        # hoist live scores+exp ahead of AV consumption
                        budget = 2
                        while budget > 0 and sc_emitted < len(blocks) and \
                                (sc_emitted - av_i) < HOIST_GAP:
                            emit_sc_exp(sc_emitted)
                            sc_emitted += 1
                            budget -= 1
                        while sc_emitted <= av_i:
                            emit_sc_exp(sc_emitted)
                            sc_emitted += 1

                        r = jb - 4 * c
                        off = 128 * r if r > 0 else 0
                        ex = ex_tiles.pop(av_i)
                        vb = p * 193
                        nc.tensor.matmul(
                            av_e[0:65, off:512],
                            lhsT=v_t[jb][:, vb:vb + 65],
                            rhs=ex[:, off:512],
                            start=(jb == 0), stop=(jb == njb - 1),
                            skip_group_check=True,
                        )
                        nc.tensor.matmul(
                            av_o[:, off:512],
                            lhsT=v_t[jb][:, vb + 65:vb + 193],
                            rhs=ex[:, 512 + off:1024],
                            start=(jb == 0), stop=(jb == njb - 1),
                            skip_group_check=True,
                        )
                        av_i += 1
                        if jb == 2 and pending_norm:
                            flush_norm()

                    # recip of the sumexp rows now (DVE); the PE-side
                    # broadcast + the muls are deferred into the next unit
                    # reciprocal via stream-transpose: tokens land on
                    # partitions, so the builtin (iterative) reciprocal sees
                    # only 16 elems/partition instead of 512 (3.3us -> 0.1us).
                    tr = norm_pool.tile([64, 512], f32, tag="tr", name="tr")
                    rt = norm_pool.tile([64, 512], f32, tag="rt", name="rt")
                    rec2 = norm_pool.tile([64, 1024], bf16, tag="rec2",
                                          name="rec2")
                    nc.vector.tensor_copy(st_p[0:1, :], av_e[64:65, 0:512])
                    nc.vector.tensor_copy(st_p[32:33, :], av_o[0:1, 0:512])
                    nc.vector.transpose(tr[:], st_p[:])
                    tr3 = tr.rearrange("p (k j) -> p k j", j=32)
                    rc3 = rc_p.rearrange("p (k j) -> p k j", j=32)
                    with nc.allow_low_precision(reason="softmax recip"):
                        nc.vector.reciprocal(rc3[:, :, 0:1], tr3[:, :, 0:1])
                    nc.vector.transpose(rt[:], rc_p[:])
                    nc.vector.tensor_copy(rec2[0:1, 0:512], rt[0:1, :])
                    nc.vector.tensor_copy(rec2[32:33, 512:1024], rt[32:33, :])
                    pending_norm.append((c, p, av_e, av_o, rec2))

                    if p == 0 and 1 <= c <= NC_ - 2:
                        emit_proj(c - 1)
            emit_proj(NC_ - 2)
            while pending_norm:
                flush_norm()
            emit_proj(NC_ - 1)

    nc.compile()
    return nc


def _get_program():
    global _PROG
    if _PROG is None:
        _PROG = _build_program()
    return _PROG


def _host_inputs(x, Wqkv, Wproj):
    """Build the 8 per-core input maps (bf16 operands)."""
    import ml_dtypes

    bf16 = ml_dtypes.bfloat16
    x = np.asarray(x, np.float32)
    Wqkv = np.asarray(Wqkv, np.float32)
    Wproj = np.asarray(Wproj, np.float32)

    Wq = Wqkv[:, :D].reshape(D, H, DH)
    Wk = Wqkv[:, D:2 * D].reshape(D, H, DH)
    Wv = Wqkv[:, 2 * D:].reshape(D, H, DH)

    # causal mask: keep j <= i within the diagonal 128-block; x2 copies
    j = np.arange(128)[:, None]
    i = np.arange(128)[None, :]
    tri = (j <= i).astype(np.float32)
    mask = np.concatenate([tri, tri], axis=1).astype(bf16)  # [128, 256]

    # per jb: two 65-col halves, each [1, 1, 0*63]
    pat = np.zeros(130, np.float32)
    pat[0] = pat[1] = pat[65] = pat[66] = 1.0
    vinit = np.tile(pat, (128, NTT)).astype(bf16)

    in_maps = []
    for b in range(B):
        xT = np.ascontiguousarray(x[b].T).astype(bf16)  # [D, T]
        for g in range(4):
            hs = slice(g * HPG, (g + 1) * HPG)
            wqkv = np.concatenate(
                [Wq[:, hs].reshape(D, HPG * DH),
                 Wk[:, hs].reshape(D, HPG * DH),
                 Wv[:, hs].reshape(D, HPG * DH)], axis=1,
            ).astype(bf16)
            wp = (Wproj[g * 256:(g + 1) * 256]
                  .reshape(2, 128, D).transpose(1, 0, 2).reshape(128, 2 * D)
                  ).astype(bf16)
            in_maps.append({
                "xT": xT,
                "wqkv": np.ascontiguousarray(wqkv),
                "wproj": np.ascontiguousarray(wp),
                "mask": mask,
                "vinit": vinit,
                "ones": np.ones((128, 128), bf16),
            })
    return in_maps


def kernel(x, Wqkv, Wproj):
    from concourse.bass_utils import run_bass_kernel_spmd

    nc = _get_program()
    in_maps = _host_inputs(x, Wqkv, Wproj)
    res = run_bass_kernel_spmd(nc, in_maps, core_ids=list(range(8)))
    outs = [np.asarray(r["out"], dtype=np.float32) for r in res.results]
    full = np.stack(
        [outs[b * 4] + outs[b * 4 + 1] + outs[b * 4 + 2] + outs[b * 4 + 3]
         for b in range(B)]
    ).astype(np.float32)
    return full


# revision 52
# speedup vs baseline: 1.0184x; 1.0184x over previous
"""Causal self-attention (B=2, T=2048, D=1024, H=16) on 8 trn2 NeuronCores.

Sharding: core = b*4 + g  (b = batch 0/1, g = head-group of 4 heads).
Each core computes its 4 heads' attention for its batch plus the partial
output projection (Wproj rows for those heads); host sums the 4 bf16
partials per batch (the tensor-parallel all-reduce).

Design (289us v1 baseline -> 175us):
  - all matmul operands bf16 (host ships bf16): halves input DMA and
    avoids the f32r 4-cyc/row penalty on small-free-dim diagonal blocks.
  - stage-1 x DMA is (chunk, kt)-sliced so QKV matmuls start after
    ~2.5MB instead of ~11MB.
  - softmax exp is the scalar(ACT)-engine drumbeat ((N+352)/1.2 ns per
    call, ~75us total): exp for the first NPRE blocks is pre-computed
    during stage 1 (the scalar engine idles there otherwise), and the
    remaining live blocks' scores+exp are hoisted ahead of their AV
    consumption so the scalar engine never idles and the in-order PE
    queue never waits on a just-issued exp. ex tiles live in an
    EXB-deep bf16 ring.
  - softmax normalize: sumexp rows are staged to SBUF, stream-transposed
    (32x32 blocks) so tokens land on partitions, builtin reciprocal runs
    on a [64,16] strided view (16 elems/partition: 0.1us vs 3.3us for
    the [1,512] row shape - DVE reciprocal cost is per-partition-elems),
    transposed back, cast to bf16, then broadcast to 128 partitions via
    a K=1 ones-matmul (PE) and applied by DVE muls. The PE-side
    broadcast+muls are DEFERRED into the next (c,p) unit's block stream
    so the PE never stalls on the DVE chain. (gpsimd partition_broadcast
    and the custom-DVE reciprocal_approx_fast both corrupt on HW when
    mixed with builtin ops - verified; both avoided.)
  - proj PSUM tiles share the scores tag so everything fits in 8 banks:
    stage1 = pq(2) + pv(2) + sc(4); attention = sc(4) + av(4).
  - proj output tiles are cast to bf16 on the scalar engine (ACT Copy
    needs no table) and DMA'd as bf16 partials.

Per-core layouts:
  xT      [1024, 2048] bf16  x[b] transposed (host)    -> sbuf xt  [128, 8*2048]
  wqkv    [1024, 768]  bf16  [q 4h | k 4h | v 4h] cols -> sbuf w   [128, 8*768]
  wproj   [128, 2048]  bf16  pair-major Wproj rows
  mask    [128, 256]   bf16  causal tri x2 copies
  out     [2048, 1024] bf16  partial projection output

qT/kT pair tiles [128, 512] per (qk,p,c): head-even rows 0:64, head-odd
rows 64:128 (K=64 matmuls at base partition 0/64).

v tile per key-block jb is [128, 386]:
  even pair half (65 cols):  [v_h (64) | ones (1)]        -> AV rows 0:64, sumexp row 64
  odd  pair half (128 cols): [ones | zeros*63 | v_h (64)] -> sumexp row 0, AV rows 64:128
"""

import numpy as np

B, T, D, H, DH = 2, 2048, 1024, 16, 64
HPG = 4          # heads per group (per core)
NKT = D // 128   # 8 contraction tiles over D
NTT = T // 128   # 16 tiles over T (also key blocks)
NC_ = 4          # 4 i-chunks of 512 queries
VS = 386         # per-jb v-tile stride: 65 + 128 + 65 + 128
SCALE = 1.0 / np.sqrt(DH)

EXB = 43         # ex-tile ring depth (bf16 [128,1024] tiles, 2KB/partition)
NPRE = 37        # blocks pre-scored+exp'd during stage 1 (must be < EXB)
HOIST_GAP = EXB - 6

_PROG = None


def _build_program():
    from contextlib import ExitStack
    from concourse import bacc, mybir, tile

    f32 = mybir.dt.float32
    bf16 = mybir.dt.bfloat16
    Exp = mybir.ActivationFunctionType.Exp

    nc = bacc.Bacc(
        "TRN2", target_bir_lowering=False, debug=False, enable_asserts=False,
        num_devices=8,
    )
    xT_d = nc.dram_tensor("xT", [D, T], bf16, kind="ExternalInput").ap()
    wqkv_d = nc.dram_tensor("wqkv", [D, 3 * HPG * DH], bf16, kind="ExternalInput").ap()
    wproj_d = nc.dram_tensor("wproj", [128, 2 * D], bf16, kind="ExternalInput").ap()
    mask_d = nc.dram_tensor("mask", [128, 256], bf16, kind="ExternalInput").ap()
    vinit_d = nc.dram_tensor("vinit", [128, NTT * 130], bf16, kind="ExternalInput").ap()
    ones_d = nc.dram_tensor("ones", [128, 128], bf16, kind="ExternalInput").ap()
    out_d = nc.dram_tensor("out", [T, D], bf16, kind="ExternalOutput").ap()

    # global block order: (c, p, jb) — ex-ring consumption order
    blocks = [(c, p, jb)
              for c in range(NC_) for p in range(2) for jb in range(4 * c + 4)]

    with tile.TileContext(nc) as tc, ExitStack() as ctx:
        # ---- persistent pools -------------------------------------------
        const_pool = ctx.enter_context(tc.tile_pool(name="const", bufs=1))
        qk_pool = ctx.enter_context(tc.tile_pool(name="qk", bufs=1))
        v_pool = ctx.enter_context(tc.tile_pool(name="v", bufs=1))
        att_pool = ctx.enter_context(tc.tile_pool(name="att", bufs=1))
        exp_pool = ctx.enter_context(tc.tile_pool(name="exp", bufs=EXB))
        norm_pool = ctx.enter_context(tc.tile_pool(name="norm", bufs=2))
        ot_pool = ctx.enter_context(tc.tile_pool(name="ot", bufs=2))
        # scores psum: [128,1024] f32 = 2 banks, bufs=2 -> 4 banks.
        # proj pp tiles borrow this tag.
        psc_pool = ctx.enter_context(tc.tile_pool(name="psc", bufs=2, space="PSUM"))

        mask_sb = const_pool.tile([128, 256], bf16, tag="mask")
        wproj_sb = const_pool.tile([128, 2 * D], bf16, tag="wproj")
        ones_sb = const_pool.tile([128, 128], bf16, tag="ones")
        # persistent staging for the transpose-recip (filler rows must be
        # initialized once: the stream transposes read the full tile)
        st_p = const_pool.tile([64, 512], f32, tag="stp")
        rc_p = const_pool.tile([64, 512], f32, tag="rcp")
        nc.vector.memset(st_p[:], 1.0)
        nc.vector.memset(rc_p[:], 1.0)

        qk_t = {}
        for qk in range(2):
            for p in range(2):
                for c in range(NC_):
                    qk_t[qk, p, c] = qk_pool.tile(
                        [128, 512], bf16, tag=f"qk{qk}{p}{c}",
                        name=f"qkt{qk}{p}{c}")
        v_t = [v_pool.tile([128, VS], bf16, tag=f"v{jb}", name=f"vt{jb}")
               for jb in range(NTT)]
        att_t = {}
        for p in range(2):
            for c in range(NC_):
                att_t[p, c] = att_pool.tile([128, 512], bf16, tag=f"att{p}{c}",
                                            name=f"attt{p}{c}")

        # ---- helpers -----------------------------------------------------
        ex_tiles = {}     # block index -> ex tile
        sc_emitted = 0    # blocks whose scores+exp are emitted

        mk2 = mask_sb.rearrange("p (h i) -> p h i", h=2)

        def emit_sc_exp(bi):
            """scores matmul pair + exp (+ causal mask) for block bi."""
            c, p, jb = blocks[bi]
            r = jb - 4 * c
            off = 128 * r if r > 0 else 0
            sc = psc_pool.tile([128, 1024], f32, tag="sc", name="sc")
            kt_tile = qk_t[1, p, jb // 4]
            q_tile = qk_t[0, p, c]
            for par in range(2):
                rows = slice(par * 64, par * 64 + 64)
                nc.tensor.matmul(
                    sc[:, par * 512 + off:par * 512 + 512],
                    lhsT=kt_tile[rows, (jb % 4) * 128:(jb % 4) * 128 + 128],
                    rhs=q_tile[rows, off:512],
                    start=True, stop=True,
                )
            ex = exp_pool.tile([128, 1024], bf16, tag="ex", name="ex")
            sc2 = sc.rearrange("p (h i) -> p h i", h=2)
            ex2 = ex.rearrange("p (h i) -> p h i", h=2)
            nc.scalar.activation(ex2[:, :, off:512], sc2[:, :, off:512],
                                 Exp, scale=float(SCALE))
            if r >= 0:
                nc.gpsimd.tensor_mul(
                    ex2[:, :, off:off + 128],
                    ex2[:, :, off:off + 128], mk2[:],
                )
            ex_tiles[bi] = ex

        # ---- stage 1: QKV projection with interleaved pre-scores ---------
        with (
            tc.tile_pool(name="xt", bufs=1) as xt_pool,
            tc.tile_pool(name="wq", bufs=1) as wq_pool,
            tc.tile_pool(name="pq", bufs=2, space="PSUM") as pq_pool,
            tc.tile_pool(name="pv", bufs=2, space="PSUM") as pv_pool,
        ):
            xt_sb = xt_pool.tile([128, NKT * T], bf16, tag="xt")
            w_sb = wq_pool.tile([128, NKT * 768], bf16, tag="w")
            vst = xt_pool.tile([128, NTT * 130], bf16, tag="vst")

            # DMA order: mask first (pre-block masks), w + x(c0) slices
            # interleaved, then vinit/wproj, then x(c1..c3).
            nc.sync.dma_start(mask_sb[:], mask_d[:])
            # weights ride the Activation HWDGE queue family, x rides SP:
            # the critical first-chunk set transfers on both in parallel
            for kt in range(NKT):
                nc.scalar.dma_start(
                    w_sb[:, kt * 768:(kt + 1) * 768],
                    wqkv_d[kt * 128:(kt + 1) * 128, :],
                )
                nc.sync.dma_start(
                    xt_sb[:, kt * T:kt * T + 512],
                    xT_d[kt * 128:(kt + 1) * 128, 0:512],
                )
            nc.scalar.dma_start(vst[:], vinit_d[:])
            nc.scalar.dma_start(wproj_sb[:], wproj_d[:])
            nc.scalar.dma_start(ones_sb[:], ones_d[:])
            for c in range(1, NC_):
                for kt in range(NKT):
                    nc.sync.dma_start(
                        xt_sb[:, kt * T + c * 512:kt * T + (c + 1) * 512],
                        xT_d[kt * 128:(kt + 1) * 128, c * 512:(c + 1) * 512],
                    )

            # v-tile static cols (ones/zeros) from vinit
            vst3 = vst.rearrange("p (j q y) -> p j q y", j=NTT, q=2)
            for jb in range(NTT):
                vt2 = v_t[jb].rearrange("p (q y) -> p q y", q=2)
                nc.gpsimd.tensor_copy(vt2[:, :, 64:129], vst3[:, jb, :, :])

            # q/k projection, c-major; 1 pre-block per psum tile
            for c in range(NC_):
                for p in range(2):
                    for qk in range(2):
                        ps = pq_pool.tile([128, 512], f32, tag="pq")
                        for kt in range(NKT):
                            nc.tensor.matmul(
                                ps[:],
                                lhsT=w_sb[:, kt * 768 + qk * 256 + p * 128:
                                          kt * 768 + qk * 256 + p * 128 + 128],
                                rhs=xt_sb[:, kt * T + c * 512:
                                          kt * T + c * 512 + 512],
                                start=(kt == 0), stop=(kt == NKT - 1),
                            )
                        nc.vector.tensor_copy(qk_t[qk, p, c][:], ps[:])
                        # pre-blocks of chunk <= c-1 are safe (their q/k
                        # copies are emitted); up to 2 per pq tile keeps
                        # the scalar engine fed without outpacing it
                        n = 0
                        while (sc_emitted < NPRE and n < 2
                               and blocks[sc_emitted][0] < c):
                            emit_sc_exp(sc_emitted)
                            sc_emitted += 1
                            n += 1

            # v projection: 2 token-tiles per [128,512] psum tile;
            # 2 pre-blocks per pair
            for tp in range(NTT // 2):
                ps = pv_pool.tile([128, 512], f32, tag="pv")
                for half in range(2):
                    tt = 2 * tp + half
                    for kt in range(NKT):
                        nc.tensor.matmul(
                            ps[:, half * 256:half * 256 + 256],
                            lhsT=xt_sb[:, kt * T + tt * 128:kt * T + tt * 128 + 128],
                            rhs=w_sb[:, kt * 768 + 512:kt * 768 + 768],
                            start=(kt == 0), stop=(kt == NKT - 1),
                        )
                for half in range(2):
                    tt = 2 * tp + half
                    for p in range(2):
                        base = p * 193
                        nc.vector.tensor_copy(
                            v_t[tt][:, base:base + 64],
                            ps[:, half * 256 + (2 * p) * 64:
                               half * 256 + (2 * p) * 64 + 64],
                        )
                        nc.vector.tensor_copy(
                            v_t[tt][:, base + 129:base + 193],
                            ps[:, half * 256 + (2 * p + 1) * 64:
                               half * 256 + (2 * p + 1) * 64 + 64],
                        )
                n = 0
                while sc_emitted < NPRE and n < 2:
                    emit_sc_exp(sc_emitted)
                    sc_emitted += 1
                    n += 1

        # ---- stage 2+3: attention with hoisted scores + interleaved proj -
        def emit_proj(c):
            for tt in range(4 * c, 4 * c + 4):
                pp = psc_pool.tile([128, 1024], f32, tag="sc", name="pp")
                for ch in range(2):
                    for p in range(2):
                        nc.tensor.matmul(
                            pp[:, ch * 512:ch * 512 + 512],
                            lhsT=att_t[p, tt // 4][:, (tt % 4) * 128:
                                                   (tt % 4) * 128 + 128],
                            rhs=wproj_sb[:, p * D + ch * 512:p * D + ch * 512 + 512],
                            start=(p == 0), stop=(p == 1),
                        )
                ot = ot_pool.tile([128, D], bf16, tag="ot", bufs=3)
                nc.scalar.copy(ot[:], pp[:])
                # alternate the two HWDGE queue families so output stores
                # don't serialize behind one queue's backlog
                eng = nc.sync if tt % 2 == 0 else nc.scalar
                eng.dma_start(out_d[tt * 128:tt * 128 + 128, :], ot[:])

        with tc.tile_pool(name="pav", bufs=1, space="PSUM") as pav_pool:
            av_i = 0
            pending_norm = []  # deferred broadcast+mul: (c, p, av_e, av_o, rec)

            def flush_norm():
                c0, p0, av_e0, av_o0, rec0 = pending_norm.pop(0)
                # broadcast 1/sumexp rows to all 128 partitions via a K=1
                # ones-matmul (PE; recip long done by the time PE gets here)
                rb = psc_pool.tile([128, 1024], f32, tag="sc", name="rb")
                nc.tensor.matmul(
                    rb[:, 0:512], lhsT=ones_sb[0:1, :],
                    rhs=rec0[0:1, 0:512], start=True, stop=True,
                )
                nc.tensor.matmul(
                    rb[:, 512:1024], lhsT=ones_sb[32:33, :],
                    rhs=rec0[32:33, 512:1024], start=True, stop=True,
                )
                # DVE reads only one PSUM operand per op: stage rb in SBUF
                rbs = norm_pool.tile([128, 1024], f32, tag="rbs", name="rbs")
                nc.vector.tensor_copy(rbs[:], rb[:])
                nc.vector.tensor_mul(
                    att_t[p0, c0][0:64, :], av_e0[0:64, :], rbs[0:64, 0:512])
                nc.vector.tensor_mul(
                    att_t[p0, c0][64:128, :], av_o0[64:128, :],
                    rbs[64:128, 512:1024])

            for c in range(NC_):
                njb = 4 * c + 4
                for p in range(2):
                    av_e = pav_pool.tile([128, 512], f32, tag=f"avE{p}",
                                         name="av_e")
                    av_o = pav_pool.tile([128, 512], f32, tag=f"avO{p}",
                                         name="av_o")
                    for jb in range(njb):
                        # hoist live scores+exp ahead of AV consumption
                        budget = 2
                        while budget > 0 and sc_emitted < len(blocks) and \
                                (sc_emitted - av_i) < HOIST_GAP:
                            emit_sc_exp(sc_emitted)
                            sc_emitted += 1
                            budget -= 1
                        while sc_emitted <= av_i:
                            emit_sc_exp(sc_emitted)
                            sc_emitted += 1

                        r = jb - 4 * c
                        off = 128 * r if r > 0 else 0
                        ex = ex_tiles.pop(av_i)
                        vb = p * 193
                        nc.tensor.matmul(
                            av_e[0:65, off:512],
                            lhsT=v_t[jb][:, vb:vb + 65],
                            rhs=ex[:, off:512],
                            start=(jb == 0), stop=(jb == njb - 1),
                            skip_group_check=True,
                        )
                        nc.tensor.matmul(
                            av_o[:, off:512],
                            lhsT=v_t[jb][:, vb + 65:vb + 193],
                            rhs=ex[:, 512 + off:1024],
                            start=(jb == 0), stop=(jb == njb - 1),
                            skip_group_check=True,
                        )
                        av_i += 1
                        if jb == 2 and pending_norm:
                            flush_norm()

                    # recip of the sumexp rows now (DVE); the PE-side
                    # broadcast + the muls are deferred into the next unit
                    # reciprocal via stream-transpose: tokens land on
                    # partitions, so the builtin (iterative) reciprocal sees
                    # only 16 elems/partition instead of 512 (3.3us -> 0.1us).
                    tr = norm_pool.tile([64, 512], f32, tag="tr", name="tr")
                    rt = norm_pool.tile([64, 512], f32, tag="rt", name="rt")
                    rec2 = norm_pool.tile([64, 1024], bf16, tag="rec2",
                                          name="rec2")
                    nc.vector.tensor_copy(st_p[0:1, :], av_e[64:65, 0:512])
                    nc.vector.tensor_copy(st_p[32:33, :], av_o[0:1, 0:512])
                    nc.vector.transpose(tr[:], st_p[:])
                    tr3 = tr.rearrange("p (k j) -> p k j", j=32)
                    rc3 = rc_p.rearrange("p (k j) -> p k j", j=32)
                    with nc.allow_low_precision(reason="softmax recip"):
                        nc.vector.reciprocal(rc3[:, :, 0:1], tr3[:, :, 0:1])
                    nc.vector.transpose(rt[:], rc_p[:])
                    nc.vector.tensor_copy(rec2[0:1, 0:512], rt[0:1, :])
                    nc.vector.tensor_copy(rec2[32:33, 512:1024], rt[32:33, :])
                    pending_norm.append((c, p, av_e, av_o, rec2))

                    if p == 0 and 1 <= c <= NC_ - 2:
                        emit_proj(c - 1)
            emit_proj(NC_ - 2)
            while pending_norm:
                flush_norm()
            emit_proj(NC_ - 1)

    nc.compile()
    return nc


def _get_program():
    global _PROG
    if _PROG is None:
        _PROG = _build_program()
    return _PROG


def _host_inputs(x, Wqkv, Wproj):
    """Build the 8 per-core input maps (bf16 operands)."""
    import ml_dtypes

    bf16 = ml_dtypes.bfloat16
    x = np.asarray(x, np.float32)
    Wqkv = np.asarray(Wqkv, np.float32)
    Wproj = np.asarray(Wproj, np.float32)

    Wq = Wqkv[:, :D].reshape(D, H, DH)
    Wk = Wqkv[:, D:2 * D].reshape(D, H, DH)
    Wv = Wqkv[:, 2 * D:].reshape(D, H, DH)

    # causal mask: keep j <= i within the diagonal 128-block; x2 copies
    j = np.arange(128)[:, None]
    i = np.arange(128)[None, :]
    tri = (j <= i).astype(np.float32)
    mask = np.concatenate([tri, tri], axis=1).astype(bf16)  # [128, 256]

    # per jb: two 65-col halves, each [1, 1, 0*63]
    pat = np.zeros(130, np.float32)
    pat[0] = pat[1] = pat[65] = pat[66] = 1.0
    vinit = np.tile(pat, (128, NTT)).astype(bf16)

    in_maps = []
    for b in range(B):
        xT = np.ascontiguousarray(x[b].T).astype(bf16)  # [D, T]
        for g in range(4):
            hs = slice(g * HPG, (g + 1) * HPG)
            wqkv = np.concatenate(
                [Wq[:, hs].reshape(D, HPG * DH),
                 Wk[:, hs].reshape(D, HPG * DH),
                 Wv[:, hs].reshape(D, HPG * DH)], axis=1,
            ).astype(bf16)
            wp = (Wproj[g * 256:(g + 1) * 256]
                  .reshape(2, 128, D).transpose(1, 0, 2).reshape(128, 2 * D)
                  ).astype(bf16)
            in_maps.append({
                "xT": xT,
                "wqkv": np.ascontiguousarray(wqkv),
                "wproj": np.ascontiguousarray(wp),
                "mask": mask,
                "vinit": vinit,
                "ones": np.ones((128, 128), bf16),
            })
    return in_maps


def kernel(x, Wqkv, Wproj):
    from concourse.bass_utils import run_bass_kernel_spmd

    nc = _get_program()
    in_maps = _host_inputs(x, Wqkv, Wproj)
    res = run_bass_kernel_spmd(nc, in_maps, core_ids=list(range(8)))
    outs = [np.asarray(r["out"], dtype=np.float32) for r in res.results]
    full = np.stack(
        [outs[b * 4] + outs[b * 4 + 1] + outs[b * 4 + 2] + outs[b * 4 + 3]
         for b in range(B)]
    ).astype(np.float32)
    return full
